# revision 4
# baseline (speedup 1.0000x reference)
"""ModePool2d (K=3, S=2, P=1, 17 bins) Trainium2 Bass kernel.

Input  x: (8, 64, 224, 224) f32 in [0,1).
Output  : (8, 64, 112, 112) f32 = argmax-bin/16 of the 17-bin histogram
          (bin = round-half-even(16x) clipped to [0,16]) over each 3x3
          stride-2 window of the zero-padded image, first-max tie-break.

Sharding: pure data-parallel over batch; core k handles batch k
(64 channel-images). Per-core layout: partition p = 2*c + s where
s in {0,1} selects the top/bottom half of the padded image rows, so all
128 partitions are used. Host pads/halves the input and re-assembles the
output.

Algorithm per 3x3 window: for each bin b, count via separable 3-sums of
the one-hot plane (exact integer fp32 arithmetic), score = count +
(17-b)/64 (tie-break bias: smaller bin wins ties, matching first-argmax),
running max over bins, then decode bin from the fractional part of the
max score. The rounding to bins uses the fp32 magic-number trick
(y + 2^23 rounds y to the nearest integer, half-to-even) which matches
jnp.round bit-exactly.
"""

import numpy as np

import concourse.bass as bass
import concourse.mybir as mybir
import concourse.tile as tile
from concourse import bacc
from concourse.bass_utils import run_bass_kernel_spmd

F32 = mybir.dt.float32
MAGIC = float(2 ** 23)

B, C, H, W = 8, 64, 224, 224
NCORES = 8
HO, WO = 112, 112
HHALF = 113          # padded rows per half-image
WP = 226             # padded width
ROWS_PER_HALF = 56   # output rows per half
CHUNK_OUT = 8        # output rows per chunk
RIN = 2 * CHUNK_OUT + 1  # input rows per chunk
NCHUNK = ROWS_PER_HALF // CHUNK_OUT  # 7
P = 128

_CACHE: dict = {}


def _build_program() -> bass.Bass:
    nc = bacc.Bacc("TRN2", target_bir_lowering=False, debug=False)
    x_d = nc.dram_tensor("xin", [P, HHALF, WP], F32, kind="ExternalInput")
    o_d = nc.dram_tensor("out", [P, ROWS_PER_HALF, WO], F32, kind="ExternalOutput")

    add = mybir.AluOpType.add
    AT = mybir.AluOpType

    with tile.TileContext(nc) as tc:
        from contextlib import ExitStack
        with ExitStack() as ctx:
            xpool = ctx.enter_context(tc.tile_pool(name="x", bufs=2))
            qpool = ctx.enter_context(tc.tile_pool(name="q", bufs=2))
            epool = ctx.enter_context(tc.tile_pool(name="e", bufs=2))
            hpool = ctx.enter_context(tc.tile_pool(name="h", bufs=2))
            spool = ctx.enter_context(tc.tile_pool(name="s", bufs=3))
            opool = ctx.enter_context(tc.tile_pool(name="o", bufs=2))

            for ch in range(NCHUNK):
                r0 = 2 * CHUNK_OUT * ch  # first padded row of this chunk
                X = xpool.tile([P, RIN, WP], F32)
                nc.sync.dma_start(
                    X[:, :, :],
                    bass.AP(x_d, r0 * WP,
                            [[HHALF * WP, P], [WP, RIN], [1, WP]]))
                qb = qpool.tile([P, RIN, WP], F32)
                nc.vector.tensor_scalar(qb[:, :, :], X[:, :, :], 16.0, MAGIC,
                                        AT.mult, add)

                m = spool.tile([P, CHUNK_OUT, WO], F32)

                def colsAP(t, start, step, count):
                    return bass.AP(t.tensor, start,
                                   [[RIN * WP, P], [WP, RIN], [step, count]])

                def rowsAP(t, start, step, count, width):
                    return bass.AP(t.tensor, start * width,
                                   [[RIN * width, P], [step * width, count],
                                    [1, width]])

                for b in range(17):
                    e = epool.tile([P, RIN, WP], F32)
                    nc.vector.tensor_scalar(e[:, :, :], qb[:, :, :],
                                            MAGIC + b, None, AT.is_equal)
                    h = hpool.tile([P, RIN, WO], F32)
                    nc.vector.tensor_tensor(h[:, :, :], colsAP(e, 0, 2, WO),
                                            colsAP(e, 1, 2, WO), add)
                    nc.vector.tensor_tensor(h[:, :, :], h[:, :, :],
                                            colsAP(e, 2, 2, WO), add)
                    v = spool.tile([P, CHUNK_OUT, WO], F32)
                    nc.vector.tensor_tensor(v[:, :, :],
                                            rowsAP(h, 0, 2, CHUNK_OUT, WO),
                                            rowsAP(h, 1, 2, CHUNK_OUT, WO), add)
                    s = spool.tile([P, CHUNK_OUT, WO], F32)
                    nc.vector.scalar_tensor_tensor(
                        s[:, :, :], v[:, :, :], (17.0 - b) / 64.0,
                        rowsAP(h, 2, 2, CHUNK_OUT, WO), op0=add, op1=add)
                    if b == 0:
                        nc.vector.tensor_copy(m[:, :, :], s[:, :, :])
                    else:
                        nc.vector.tensor_tensor(m[:, :, :], m[:, :, :],
                                                s[:, :, :], AT.max)

                # decode: c = round_even(m); out = 17/16 - 4*(m - c)
                cc = spool.tile([P, CHUNK_OUT, WO], F32)
                nc.vector.tensor_scalar(cc[:, :, :], m[:, :, :], MAGIC, -MAGIC,
                                        add, add)
                d = spool.tile([P, CHUNK_OUT, WO], F32)
                nc.vector.tensor_tensor(d[:, :, :], m[:, :, :], cc[:, :, :],
                                        AT.subtract)
                ot = opool.tile([P, CHUNK_OUT, WO], F32)
                nc.vector.tensor_scalar(ot[:, :, :], d[:, :, :], -4.0,
                                        17.0 / 16.0, AT.mult, add)
                nc.sync.dma_start(
                    bass.AP(o_d, ch * CHUNK_OUT * WO,
                            [[ROWS_PER_HALF * WO, P], [WO, CHUNK_OUT], [1, WO]]),
                    ot[:, :, :])
    nc.compile()
    return nc


def _host_prep(x: np.ndarray) -> np.ndarray:
    """(8,64,224,224) -> (8,128,113,226) padded half-images."""
    xp = np.zeros((B, C, 2, HHALF, WP), dtype=np.float32)
    xp[:, :, 0, 1:113, 1:225] = x[:, :, 0:112, :]
    xp[:, :, 1, 0:113, 1:225] = x[:, :, 111:224, :]
    return xp.reshape(B, P, HHALF, WP)


def kernel(x: np.ndarray) -> np.ndarray:
    x = np.asarray(x, dtype=np.float32)
    assert x.shape == (B, C, H, W)
    if "nc" not in _CACHE:
        _CACHE["nc"] = _build_program()
    nc = _CACHE["nc"]
    xp = _host_prep(x)
    in_maps = [{"xin": np.ascontiguousarray(xp[k])} for k in range(NCORES)]
    res = run_bass_kernel_spmd(nc, in_maps, core_ids=list(range(NCORES)))
    out = np.empty((B, C, HO, WO), dtype=np.float32)
    for k in range(NCORES):
        out[k] = res.results[k]["out"].reshape(C, HO, WO)
    return out


# revision 5
# speedup vs baseline: 10258.2168x; 10258.2168x over previous
"""ModePool2d (K=3, S=2, P=1, 17 bins) Trainium2 Bass kernel.

Input  x: (8, 64, 224, 224) f32 in [0,1).
Output  : (8, 64, 112, 112) f32 = argmax-bin/16 of the 17-bin histogram
          (bin = round-half-even(16x) clipped to [0,16]) over each 3x3
          stride-2 window of the zero-padded image, first-max tie-break.

Sharding: pure data-parallel over batch; core k handles batch k
(64 channel-images). Per-core layout: partition p = 2*c + s where
s in {0,1} selects the top/bottom half of the padded image rows, so all
128 partitions are used. Host pads/halves the input and re-assembles the
output.

Algorithm per 3x3 window: for each bin b, count via separable 3-sums of
the one-hot plane (exact integer fp32 arithmetic), score = count +
(17-b)/64 (tie-break bias: smaller bin wins ties, matching first-argmax),
running max over bins, then decode bin from the fractional part of the
max score. The rounding to bins uses the fp32 magic-number trick
(y + 2^23 rounds y to the nearest integer, half-to-even) which matches
jnp.round bit-exactly.
"""

import numpy as np

import concourse.bass as bass
import concourse.mybir as mybir
import concourse.tile as tile
from concourse import bacc
from concourse.bass_utils import run_bass_kernel_spmd

F32 = mybir.dt.float32
MAGIC = float(2 ** 23)

B, C, H, W = 8, 64, 224, 224
NCORES = 8
HO, WO = 112, 112
HHALF = 113          # padded rows per half-image
WP = 226             # padded width
ROWS_PER_HALF = 56   # output rows per half
CHUNK_OUT = 8        # output rows per chunk
RIN = 2 * CHUNK_OUT + 1  # input rows per chunk
NCHUNK = ROWS_PER_HALF // CHUNK_OUT  # 7
P = 128

_CACHE: dict = {}


def _build_program(repeat: int = 1) -> bass.Bass:
    nc = bacc.Bacc("TRN2", target_bir_lowering=False, debug=False)
    x_d = nc.dram_tensor("xin", [P, HHALF, WP], F32, kind="ExternalInput")
    o_d = nc.dram_tensor("out", [P, ROWS_PER_HALF, WO], F32, kind="ExternalOutput")

    add = mybir.AluOpType.add
    AT = mybir.AluOpType

    with tile.TileContext(nc) as tc:
        from contextlib import ExitStack
        with ExitStack() as ctx:
            xpool = ctx.enter_context(tc.tile_pool(name="x", bufs=2))
            qpool = ctx.enter_context(tc.tile_pool(name="q", bufs=2))
            epool = ctx.enter_context(tc.tile_pool(name="e", bufs=2))
            hpool = ctx.enter_context(tc.tile_pool(name="h", bufs=2))
            spool = ctx.enter_context(tc.tile_pool(name="s", bufs=3))
            opool = ctx.enter_context(tc.tile_pool(name="o", bufs=2))

            for ch in [c for _ in range(repeat) for c in range(NCHUNK)]:
                r0 = 2 * CHUNK_OUT * ch  # first padded row of this chunk
                X = xpool.tile([P, RIN, WP], F32)
                nc.sync.dma_start(
                    X[:, :, :],
                    bass.AP(x_d, r0 * WP,
                            [[HHALF * WP, P], [WP, RIN], [1, WP]]))
                qb = qpool.tile([P, RIN, WP], F32)
                nc.vector.tensor_scalar(qb[:, :, :], X[:, :, :], 16.0, MAGIC,
                                        AT.mult, add)

                m = spool.tile([P, CHUNK_OUT, WO], F32)

                def colsAP(t, start, step, count):
                    return bass.AP(t.tensor, start,
                                   [[RIN * WP, P], [WP, RIN], [step, count]])

                def rowsAP(t, start, step, count, width):
                    return bass.AP(t.tensor, start * width,
                                   [[RIN * width, P], [step * width, count],
                                    [1, width]])

                for b in range(17):
                    e = epool.tile([P, RIN, WP], F32)
                    nc.vector.tensor_scalar(e[:, :, :], qb[:, :, :],
                                            MAGIC + b, None, AT.is_equal)
                    h = hpool.tile([P, RIN, WO], F32)
                    nc.vector.tensor_tensor(h[:, :, :], colsAP(e, 0, 2, WO),
                                            colsAP(e, 1, 2, WO), add)
                    nc.vector.tensor_tensor(h[:, :, :], h[:, :, :],
                                            colsAP(e, 2, 2, WO), add)
                    v = spool.tile([P, CHUNK_OUT, WO], F32)
                    nc.vector.tensor_tensor(v[:, :, :],
                                            rowsAP(h, 0, 2, CHUNK_OUT, WO),
                                            rowsAP(h, 1, 2, CHUNK_OUT, WO), add)
                    s = spool.tile([P, CHUNK_OUT, WO], F32)
                    nc.vector.scalar_tensor_tensor(
                        s[:, :, :], v[:, :, :], (17.0 - b) / 64.0,
                        rowsAP(h, 2, 2, CHUNK_OUT, WO), op0=add, op1=add)
                    if b == 0:
                        nc.vector.tensor_copy(m[:, :, :], s[:, :, :])
                    else:
                        nc.vector.tensor_tensor(m[:, :, :], m[:, :, :],
                                                s[:, :, :], AT.max)

                # decode: c = round_even(m); out = 17/16 - 4*(m - c)
                cc = spool.tile([P, CHUNK_OUT, WO], F32)
                nc.vector.tensor_scalar(cc[:, :, :], m[:, :, :], MAGIC, -MAGIC,
                                        add, add)
                d = spool.tile([P, CHUNK_OUT, WO], F32)
                nc.vector.tensor_tensor(d[:, :, :], m[:, :, :], cc[:, :, :],
                                        AT.subtract)
                ot = opool.tile([P, CHUNK_OUT, WO], F32)
                nc.vector.tensor_scalar(ot[:, :, :], d[:, :, :], -4.0,
                                        17.0 / 16.0, AT.mult, add)
                nc.sync.dma_start(
                    bass.AP(o_d, ch * CHUNK_OUT * WO,
                            [[ROWS_PER_HALF * WO, P], [WO, CHUNK_OUT], [1, WO]]),
                    ot[:, :, :])
    nc.compile()
    return nc


def _host_prep(x: np.ndarray) -> np.ndarray:
    """(8,64,224,224) -> (8,128,113,226) padded half-images."""
    xp = np.zeros((B, C, 2, HHALF, WP), dtype=np.float32)
    xp[:, :, 0, 1:113, 1:225] = x[:, :, 0:112, :]
    xp[:, :, 1, 0:113, 1:225] = x[:, :, 111:224, :]
    return xp.reshape(B, P, HHALF, WP)


def kernel(x: np.ndarray) -> np.ndarray:
    x = np.asarray(x, dtype=np.float32)
    assert x.shape == (B, C, H, W)
    if "nc" not in _CACHE:
        _CACHE["nc"] = _build_program()
    nc = _CACHE["nc"]
    xp = _host_prep(x)
    in_maps = [{"xin": np.ascontiguousarray(xp[k])} for k in range(NCORES)]
    res = run_bass_kernel_spmd(nc, in_maps, core_ids=list(range(NCORES)))
    out = np.empty((B, C, HO, WO), dtype=np.float32)
    for k in range(NCORES):
        out[k] = res.results[k]["out"].reshape(C, HO, WO)
    return out


# revision 6
# speedup vs baseline: 10277.5140x; 1.0019x over previous
"""ModePool2d (K=3, S=2, P=1, 17 bins) Trainium2 Bass kernel.

Input  x: (8, 64, 224, 224) f32 in [0,1).
Output  : (8, 64, 112, 112) f32 = argmax-bin/16 of the 17-bin histogram
(bin = round-half-even(16x) in [0,16]) over each 3x3 stride-2 window of
the zero-padded image, first-max tie-break — bit-exact vs the jax
reference.

Sharding: pure data-parallel over batch; core k handles batch k (64
channel-images).  Per-core partition p = 2*c + s, s in {0,1} = top /
bottom half of the padded rows, so all 128 partitions are used.  The
host pads/halves the input and reassembles the output.

Algorithm (all exact fp32/fp16 integer arithmetic):
 * qb = 2^23 + round_half_even(16 x) via the fp32 magic-number trick
   (one tensor_scalar; matches jnp.round bit-exactly, including
   half-way ties).
 * Bins processed in pairs (b0, b1 = b0+1) with radix-64 packing.
   Custom DVE ops evaluate, per element of a 113-wide logical grid,
   pack2(r) = eq(r, 2^23+b0) + 64 * eq(r, 2^23+b1).  The three window
   column sets (stride-2 offsets 0/1/2 of qb) are 1-free-dim views, so
   the horizontal 3-sum of pack2 takes TWO custom instructions per bin
   pair (pair-pack of cols 0&1, then accumulate col 2).
 * Vertical 3-sum: one fp16 tensor_tensor add (2x packed mode) plus a
   fused custom op that adds the third row, extracts the two counts
   (round-to-64 magic), forms scores = count + (17-bin)/64 and maxes.
 * Scores max-reduced over the 9 pair groups (fp16 2x), then one fused
   custom op decodes m = c* + (17-b*)/64 into b*/16 (exact).
Ties: equal counts give the smaller bin via the (17-b)/64 bias = the
reference's first-argmax semantics; count differences (>=1) dominate
all biases (<=17/64).
"""

import numpy as np

import concourse.bass as bass
import concourse.mybir as mybir
import concourse.tile as tile
from concourse import bacc
from concourse.bass_utils import run_bass_kernel_spmd

# --------------------------------------------------------------------------
# Custom DVE ops (registered into concourse.dve_ops at import time)
# --------------------------------------------------------------------------
from concourse.dve_spec import (
    Spec, Src0, Src1, C0, C1, C2, maxx, eq, lower,
)
from concourse.dve_ops import (
    DveOp, OPS, CUSTOM_DVE_SPECS, _SUB_OPCODE_FOR_NAME, has_src1,
)
from concourse.dve_uop import DveOpSpec

MAGIC = float(2 ** 23)
K29 = float(2 ** 29)


def _pack2(r, t0, t1):
    r = np.asarray(r, dtype=np.float64)
    return ((r == t0) + 64.0 * (r == t1)).astype(np.float32)


def _ref_h1pair(in0, in1, s0, s1, imm2):
    return (_pack2(in0, s0, s1) + _pack2(in1, s0, s1)).astype(np.float32)


def _ref_p2acc(in0, in1, s0, s1, imm2):
    return (_pack2(in0, s0, s1) + np.asarray(in1, np.float32)).astype(np.float32)


def _ref_dec2v(in0, in1, s0, s1, imm2):
    h2 = np.asarray(in0, np.float64).reshape(in0.shape[0], -1)
    v1 = np.asarray(in1, np.float64).reshape(in1.shape[0], -1)
    v = h2 + v1
    a = np.round(v / 64) * 64  # n0 <= 9 -> round == floor
    return np.maximum((v - a) + s1, a * imm2 + (s1 - imm2)).astype(np.float32)


def _ref_findec(in0, in1, s0, s1, imm2):
    m = np.asarray(in0, np.float64)
    return ((m - np.round(m)) * s1 + imm2).astype(np.float32)


MP_H1PAIR_BODY = (eq(Src0, C0) + eq(Src1, C0)) + \
                 (eq(Src0, C1) + eq(Src1, C1)) * C2
MP_P2ACC_BODY = eq(Src0, C0) + eq(Src0, C1) * C2 + Src1
_vv = Src0 + Src1
_av = (_vv + C0) - C0
MP_DEC2V_BODY = maxx((_vv - _av) + C1, _av * C2 + (C1 - C2))
_rr = (Src0 + C0) - C0
MP_FINDEC_BODY = (Src0 - _rr) * C1 + C2


def _make_op(name, body, reference):
    existing = {op.name: op for op in OPS}
    if name in existing:           # idempotent across re-imports
        return existing[name]
    spec = Spec(body=body, reference=reference)
    opcode = max(_SUB_OPCODE_FOR_NAME.values()) + 1
    shas = {}
    for ver in ("v3", "v4"):
        uops = lower(spec, ver=ver)
        tmp = DveOpSpec(name=name, opcode=opcode, uops=uops,
                        rd1_en=has_src1(spec))
        shas[ver] = tmp.sha(ver)
    op = DveOp(name, spec, subdim=False, uops_sha=shas)
    OPS.append(op)
    CUSTOM_DVE_SPECS[name] = spec
    _SUB_OPCODE_FOR_NAME[name] = opcode
    return op


MP_H1PAIR = _make_op("MP_H1PAIR", MP_H1PAIR_BODY, _ref_h1pair)
MP_P2ACC = _make_op("MP_P2ACC", MP_P2ACC_BODY, _ref_p2acc)
MP_DEC2V = _make_op("MP_DEC2V", MP_DEC2V_BODY, _ref_dec2v)
MP_FINDEC = _make_op("MP_FINDEC", MP_FINDEC_BODY, _ref_findec)

# --------------------------------------------------------------------------
# Kernel
# --------------------------------------------------------------------------
F32 = mybir.dt.float32
F16 = mybir.dt.float16

B, C, H, W = 8, 64, 224, 224
NCORES = 8
HO, WO = 112, 112
HHALF = 113          # padded rows per half-image
WP = 226             # padded width
ROWS_PER_HALF = 56
CHUNK_OUT = 14       # output rows per chunk
RIN = 2 * CHUNK_OUT + 1
NCHUNK = ROWS_PER_HALF // CHUNK_OUT
VO = CHUNK_OUT * WO
P = 128
G = RIN * 113        # 113-wide logical h grid
HS = 114             # padded h row stride (fp16 4B alignment)
GH = RIN * HS

_CACHE: dict = {}


def _build_program(repeat: int = 1) -> bass.Bass:
    nc = bacc.Bacc("TRN2", target_bir_lowering=False, debug=False)
    x_d = nc.dram_tensor("xin", [P, HHALF, WP], F32, kind="ExternalInput")
    o_d = nc.dram_tensor("out", [P, ROWS_PER_HALF, WO], F32,
                         kind="ExternalOutput")
    AT = mybir.AluOpType

    with tile.TileContext(nc) as tc:
        from contextlib import ExitStack
        with ExitStack() as ctx:
            xpool = ctx.enter_context(tc.tile_pool(name="x", bufs=2))
            qpool = ctx.enter_context(tc.tile_pool(name="q", bufs=2))
            tpool = ctx.enter_context(tc.tile_pool(name="t", bufs=2))
            hpool = ctx.enter_context(tc.tile_pool(name="h", bufs=2))
            vpool = ctx.enter_context(tc.tile_pool(name="v", bufs=2))
            mpool = ctx.enter_context(tc.tile_pool(name="m", bufs=1))
            opool = ctx.enter_context(tc.tile_pool(name="o", bufs=2))

            for _rep in range(repeat):
                for ch in range(NCHUNK):
                    r0 = 2 * CHUNK_OUT * ch
                    X = xpool.tile([P, RIN * WP + 1], F32)
                    nc.sync.dma_start(
                        X[:, 0:RIN * WP],
                        bass.AP(x_d, r0 * WP, [[HHALF * WP, P], [1, RIN * WP]]))
                    nc.vector.memset(X[:, RIN * WP:], 0.0)
                    qb = qpool.tile([P, RIN * WP + 1], F32)
                    nc.vector.tensor_scalar(qb[:, :], X[:, :], 16.0, MAGIC,
                                            AT.mult, AT.add)

                    def qview(off, n):
                        return bass.AP(qb.tensor, off,
                                       [[RIN * WP + 1, P], [2, n]])

                    m_all = mpool.tile([P, 9 * VO], F16)
                    m_list = []
                    for g in range(9):
                        b0, b1 = 2 * g, 2 * g + 1
                        t = tpool.tile([P, G], F16, tag="t")
                        nc.vector._custom_dve(
                            MP_H1PAIR, out=t[:, :],
                            in0=qview(0, G), in1=qview(1, G),
                            s0=MAGIC + b0, s1=MAGIC + b1, imm2=64.0)
                        h = hpool.tile([P, GH], F16, tag="h")
                        h2d = bass.AP(h.tensor, 0,
                                      [[GH, P], [HS, RIN], [1, 113]])
                        nc.vector._custom_dve(
                            MP_P2ACC, out=h2d,
                            in0=qview(2, G), in1=t[:, :],
                            s0=MAGIC + b0, s1=MAGIC + b1, imm2=64.0)

                        def hrows(start):
                            return bass.AP(h.tensor, start * HS,
                                           [[GH, P], [2 * HS, CHUNK_OUT],
                                            [1, WO]])

                        v1 = vpool.tile([P, VO], F16, tag="v1")
                        nc.vector.tensor_tensor(v1[:, :], hrows(0), hrows(1),
                                                AT.add)
                        mg = m_all[:, g * VO:(g + 1) * VO]
                        nc.vector._custom_dve(
                            MP_DEC2V, out=mg, in0=hrows(2), in1=v1[:, :],
                            s0=K29, s1=(17.0 - b0) / 64.0, imm2=1.0 / 64.0)
                        m_list.append(mg)

                    while len(m_list) > 1:
                        nxt = []
                        for i in range(0, len(m_list) - 1, 2):
                            a, b2 = m_list[i], m_list[i + 1]
                            nc.vector.tensor_tensor(a, a, b2, AT.max)
                            nxt.append(a)
                        if len(m_list) % 2:
                            nxt.append(m_list[-1])
                        m_list = nxt
                    m = m_list[0]

                    ot = opool.tile([P, VO], F32, tag="ot")
                    nc.vector._custom_dve(
                        MP_FINDEC, out=ot[:, :], in0=m,
                        s0=MAGIC, s1=-4.0, imm2=17.0 / 16.0)
                    nc.sync.dma_start(
                        bass.AP(o_d, ch * CHUNK_OUT * WO,
                                [[ROWS_PER_HALF * WO, P], [1, VO]]),
                        ot[:, :])
    nc.compile()
    return nc


def _host_prep(x: np.ndarray) -> np.ndarray:
    xp = np.zeros((B, C, 2, HHALF, WP), dtype=np.float32)
    xp[:, :, 0, 1:113, 1:225] = x[:, :, 0:112, :]
    xp[:, :, 1, 0:113, 1:225] = x[:, :, 111:224, :]
    return xp.reshape(B, P, HHALF, WP)


def kernel(x: np.ndarray) -> np.ndarray:
    x = np.asarray(x, dtype=np.float32)
    assert x.shape == (B, C, H, W)
    if "nc" not in _CACHE:
        _CACHE["nc"] = _build_program()
    nc = _CACHE["nc"]
    xp = _host_prep(x)
    in_maps = [{"xin": np.ascontiguousarray(xp[k])} for k in range(NCORES)]
    res = run_bass_kernel_spmd(nc, in_maps, core_ids=list(range(NCORES)))
    out = np.empty((B, C, HO, WO), dtype=np.float32)
    for k in range(NCORES):
        out[k] = res.results[k]["out"].reshape(C, HO, WO)
    return out
